# revision 4
# baseline (speedup 1.0000x reference)
"""Trainium2 Bass kernel for nn_ClsHeader (octree pooling classifier head).

Data-parallel over the batch dimension: each of the 8 NeuronCores processes
one sample (its full octree subtree), weights are replicated, outputs are
gathered host-side.  No collectives needed.

v2: the whole data path runs in bf16 (host marshals data0/1/2 and conv
weights to bf16; the 2e-2 rel-err budget absorbs the ~0.4% rounding).
This halves the per-core HBM traffic (18.25MB -> 9.1MB) and lets the
sibling max-pool run as a tensor_tensor max *tree*, which the DVE executes
in 2x_1P mode for 16-bit dtypes (tensor_reduce is capped at 1x).

Per-core pipeline:
  - data0 [32768,128] (depth 5): two 4MB bf16 DMA loads (SP / ACT HWDGE
    rings), partition = 2 contiguous depth-3 subtrees (128 rows = 32KB
    contiguous per partition); 6-level TT-max tree (64 -> 1 per subtree)
    -> s0 [128, 2, C]; PE transposes (bf16 PSUM) collect all 512 d3 nodes
    into one PSUM bank; one strided reduce folds (g, q) -> x0 [128 ch, 64].
  - data1 [4096,128] (depth 4): partition = d3 node (8 rows), 3-level tree.
  - data2 [512,128] (depth 3): partition = half-d2 (4 rows), 2-level tree,
    the final sibling pair folds after the transpose.
  - Conv1x1+BN folded host-side into W' = conv_w*inv (bf16), b' (f32).
    y^T = W'^T @ x^T as 24 bf16 matmuls (3 K-chunks x 8 M-chunks) into two
    parity-interleaved PSUM banks; ScalarE activation(Relu, bias, accum_out)
    fuses bias+relu+node-sum.
  - Head matmul: logit[1,40] = sum_m s_m^T @ (head_w/64)_m + head_b; output
    DMA via GPSIMD.

The walrus build here accepts only one sync-wait per instruction, so
_split_multiwaits() rewrites the scheduled program, moving extra waits onto
single-wait NOPs.  kernel() runs through a cached jitted shard_map executor
(the same custom-call path run_bass_kernel_spmd uses under axon) so repeated
calls do not re-trace or re-compile.
"""

import os
import sys

for _p in ("/opt/trn_rl_repo", "/root/.axon_site/_ro/trn_rl_repo"):
    if os.path.isdir(_p) and _p not in sys.path:
        sys.path.append(_p)

import numpy as np
import ml_dtypes

import concourse.bass as bass
import concourse.mybir as mybir
import concourse.tile as tile
from concourse.masks import make_identity

F32 = mybir.dt.float32
BF16 = mybir.dt.bfloat16
NPBF16 = ml_dtypes.bfloat16
N_CORES = 8
D0, D1, D2 = 32768, 4096, 512  # per-core (per-sample) rows at depths 5/4/3
C = 128  # channels per input level
OUTC = 1024  # conv output channels
NCLS = 40
N2 = 64  # depth-2 nodes per sample
AX = mybir.AxisListType.X
AXY = mybir.AxisListType.XY
MAX = mybir.AluOpType.max


def _split_multiwaits(nc):
    """The walrus build in this container accepts only ONE sync-wait per
    instruction; move extra waits onto dedicated NOPs inserted just before
    the owning instruction (same engine, so sequencer order is preserved)."""
    n_split = 0
    for f in nc.m.functions:
        for bb in f.blocks:
            out = []
            changed = False
            for inst in bb.instructions:
                si = inst.sync_info
                waits = list(si.on_wait) if si is not None else []
                if len(waits) > 1:
                    for j, w in enumerate(waits[:-1]):
                        nop = mybir.InstNoOp(
                            name=f"{inst.name}-wsplit{j}", ins=[], outs=[]
                        )
                        nop.engine = inst.engine
                        nop.sync_info = mybir.SyncInfo(on_wait=[w], on_update=[])
                        out.append(nop)
                    si.on_wait = [waits[-1]]
                    changed = True
                    n_split += 1
                out.append(inst)
            if changed:
                bb.instructions = out
    return n_split


def _tt_max_tree(nc, pool, src, g, k, tag):
    """Pool src [128, g, k, C] bf16 -> [128, g, C] via a TT-max halving tree.
    Every level is contiguous-innermost so the DVE runs 2x_1P."""
    cur, kk = src, k
    lvl = 0
    while kk > 1:
        half = kk // 2
        if half == 1:
            out = pool.tile([128, g, C], BF16, tag=f"{tag}s", bufs=2)
            nc.vector.tensor_tensor(
                out=out[:],
                in0=cur[:, :, 0, :],
                in1=cur[:, :, 1, :],
                op=MAX,
            )
        else:
            out = pool.tile([128, g, half, C], BF16, tag=f"{tag}l{lvl}", bufs=1)
            nc.vector.tensor_tensor(
                out=out[:],
                in0=cur[:, :, 0:half, :],
                in1=cur[:, :, half:kk, :],
                op=MAX,
            )
        cur, kk = out, half
        lvl += 1
    return cur


def _build_nc(split=True, repeat=1, mode="full"):
    # mode: "full" | "dma" (loads only, no compute) | "compute" (no big loads)
    nc = bass.Bass("TRN2", num_devices=N_CORES)
    d0 = nc.dram_tensor("d0", [D0, C], BF16, kind="ExternalInput")
    d1 = nc.dram_tensor("d1", [D1, C], BF16, kind="ExternalInput")
    d2 = nc.dram_tensor("d2", [D2, C], BF16, kind="ExternalInput")
    wT = nc.dram_tensor("wT", [128, 3 * OUTC], BF16, kind="ExternalInput")
    bias8 = nc.dram_tensor("bias8", [128, 8], F32, kind="ExternalInput")
    hw8 = nc.dram_tensor("hw8", [128, 8 * NCLS], F32, kind="ExternalInput")
    hb = nc.dram_tensor("hb", [1, NCLS], F32, kind="ExternalInput")
    out = nc.dram_tensor("out", [1, NCLS], F32, kind="ExternalOutput")

    with tile.TileContext(nc) as tc:
        with (
            tc.tile_pool(name="consts", bufs=1) as consts,
            tc.tile_pool(name="inp", bufs=3) as inp,
            tc.tile_pool(name="work", bufs=1) as work,
            tc.tile_pool(name="pt", bufs=1, space="PSUM") as pt,
            tc.tile_pool(name="py", bufs=1, space="PSUM") as py,
        ):
            wT_s = consts.tile([128, 3 * OUTC], BF16)
            nc.scalar.dma_start(out=wT_s[:], in_=wT[:])
            bias8_s = consts.tile([128, 8], F32)
            nc.scalar.dma_start(out=bias8_s[:], in_=bias8[:])
            hw8_s = consts.tile([128, 8 * NCLS], F32)
            nc.scalar.dma_start(out=hw8_s[:], in_=hw8[:])
            hb_s = consts.tile([1, NCLS], F32)
            nc.scalar.dma_start(out=hb_s[:], in_=hb[:])
            ident = consts.tile([128, 128], BF16)
            make_identity(nc, ident[:])
            ones1 = consts.tile([1, 1], F32)
            nc.vector.memset(ones1[:], 1.0)
            # warm-up Relu so the ACT table DMA (~2.7us) overlaps the ramp
            # instead of sitting in front of the first real activation
            actwarm = consts.tile([1, 1], F32)
            nc.scalar.activation(
                actwarm[:], ones1[:], mybir.ActivationFunctionType.Relu
            )

            if mode == "compute":
                ld1c = consts.tile([128, 4, 8, C], BF16)
                nc.vector.memset(ld1c[:], 0.25)
                ld2c = consts.tile([128, 4, C], BF16)
                nc.vector.memset(ld2c[:], 0.25)
                ldcs = []
                for t in range(2):
                    ldc = consts.tile([128, 2, 64, C], BF16, name=f"ldc{t}")
                    nc.vector.memset(ldc[:], 0.25)
                    ldcs.append(ldc)

            # DRAM views
            # d0 row = ((t*128 + p)*2 + g)*64 + k  ->  d3 node D = t*256+2p+g
            d0v = d0[:].rearrange("(t p g k) c -> t p g k c", t=2, p=128, g=2, k=64)
            # d1 row = (j*128 + p)*8 + k  ->  d3 node D = j*128+p
            d1v = d1[:].rearrange("(j p k) c -> p j k c", j=4, p=128, k=8)
            # d2 row = q*4 + k  (q = 2*d2node + h)
            d2v = d2[:].rearrange("(p k) c -> p k c", p=128, k=4)

            for _rep in range(repeat):
              x0 = work.tile([128, N2], BF16, tag="x0", bufs=2)
              x1 = work.tile([128, N2], BF16, tag="x1", bufs=2)
              x2 = work.tile([128, N2], BF16, tag="x2", bufs=2)
              stile = work.tile([128, 8], F32, tag="stile", bufs=2)
              yscr = work.tile([128, 64], F32, tag="yscr")
              outs = work.tile([1, NCLS], F32, tag="outs")

              # PSUM: tpb0 (bf16, 1KB) collects the 4 data0 transposes, tpb1
              # the 4 data1 transposes, tp2 the data2 transpose; two f32
              # banks for the conv output (ScalarE drains one while PE fills
              # the other) and one for the head logits.
              tpb0 = pt.tile([128, 512], BF16, tag="tpb0", bufs=1)
              tpb1 = pt.tile([128, 512], BF16, tag="tpb1", bufs=1)
              tp2 = pt.tile([128, 128], BF16, tag="tp2", bufs=1)
              psum_ya = py.tile([128, 256], F32, tag="pya")
              psum_yb = py.tile([128, 256], F32, tag="pyb")
              psum_l = py.tile([1, NCLS], F32, tag="pl")

              # ---- loads ----
              ld0s = []
              for t in range(2):
                  ld = ldcs[t] if mode == "compute" else inp.tile(
                      [128, 2, 64, C], BF16, tag="ld0"
                  )
                  if mode != "compute":
                      eng = nc.sync if t == 0 else nc.scalar
                      eng.dma_start(out=ld[:], in_=d0v[t])
                  ld0s.append(ld)
              ld1 = ld1c if mode == "compute" else inp.tile(
                  [128, 4, 8, C], BF16, bufs=2, tag="ld1"
              )
              ld2 = ld2c if mode == "compute" else inp.tile(
                  [128, 4, C], BF16, bufs=2, tag="ld2"
              )
              if mode != "compute":
                  nc.sync.dma_start(out=ld1[:], in_=d1v)
                  nc.scalar.dma_start(out=ld2[:], in_=d2v)

              if mode == "dma":
                  dummy = work.tile([1, 8], F32, tag="dummy")
                  for t in range(2):
                      nc.vector.tensor_copy(
                          dummy[:, t : t + 1], ld0s[t][0:1, 0, 0, 0:1]
                      )
                  nc.vector.tensor_copy(dummy[:, 4:5], ld1[0:1, 0, 0, 0:1])
                  nc.vector.tensor_copy(dummy[:, 5:6], ld2[0:1, 0, 0:1])
                  nc.vector.tensor_copy(outs[:, 0:8], dummy[:, 0:8])
                  nc.gpsimd.dma_start(out=out[:], in_=outs[:])
                  continue

              # ---- data0: TT-max tree (64 -> 1 per d3 subtree) ----
              for t in range(2):
                  s0 = _tt_max_tree(nc, work, ld0s[t][:], 2, 64, "t0")
                  for g in range(2):
                      nc.tensor.transpose(
                          tpb0[:, (t * 2 + g) * 128 : (t * 2 + g + 1) * 128],
                          s0[:, g, :],
                          ident[:],
                      )
              # cols of tpb0: t*256 + g*128 + p ; d3 node D = t*256 + 2p + g
              # x0[:, t*32+n'] = max over g,q of col (t,g,4n'+q)
              nc.vector.reduce_max(
                  x0[:].rearrange("r (t n) -> r t n", t=2),
                  tpb0[:].rearrange("r (t g n q) -> r t n g q", t=2, g=2, n=32),
                  axis=AXY,
              )

              # ---- data1: partition = d3 node (8 contiguous rows) ----
              s1 = _tt_max_tree(nc, work, ld1[:], 4, 8, "t1")
              for j in range(4):
                  nc.tensor.transpose(
                      tpb1[:, j * 128 : (j + 1) * 128], s1[:, j, :], ident[:]
                  )
              nc.vector.reduce_max(
                  x1[:], tpb1[:].rearrange("r (n k) -> r n k", k=8), axis=AX
              )

              # ---- data2: partition = half d2 node (4 contiguous rows) ----
              s2 = _tt_max_tree(nc, work, ld2[:].rearrange("p (g k) c -> p g k c", g=1), 1, 4, "t2")
              nc.tensor.transpose(tp2[:], s2[:, 0, :], ident[:])
              nc.vector.reduce_max(
                  x2[:], tp2[:].rearrange("r (n h) -> r n h", h=2), axis=AX
              )

              if mode == "pool":
                  nc.vector.tensor_copy(outs[:, 0:1], x0[0:1, 0:1])
                  nc.vector.tensor_copy(outs[:, 1:2], x1[0:1, 0:1])
                  nc.vector.tensor_copy(outs[:, 2:3], x2[0:1, 0:1])
                  nc.gpsimd.dma_start(out=out[:], in_=outs[:])
                  continue

              # Conv matmuls: 8 m-chunks x 3 K-chunks; accumulation groups
              # run back-to-back within each bank.  ScalarE activation
              # (bias + relu + node-sum via accum_out) drains each slice.
              for m in range(8):
                  pybank = psum_ya if m % 2 == 0 else psum_yb
                  sl = pybank[:, (m // 2) * 64 : (m // 2 + 1) * 64]
                  for ki, xk in ((0, x0), (1, x1), (2, x2)):
                      nc.tensor.matmul(
                          sl,
                          wT_s[:, ki * OUTC + m * 128 : ki * OUTC + (m + 1) * 128],
                          xk[:],
                          start=(ki == 0),
                          stop=(ki == 2),
                      )
                  nc.scalar.activation(
                      yscr[:],
                      sl,
                      mybir.ActivationFunctionType.Relu,
                      bias=bias8_s[:, m : m + 1],
                      scale=1.0,
                      accum_out=stile[:, m : m + 1],
                  )

              for m in range(8):
                  nc.tensor.matmul(
                      psum_l[:],
                      stile[:, m : m + 1],
                      hw8_s[:, m * NCLS : (m + 1) * NCLS],
                      start=(m == 0),
                      stop=False,
                  )
              # head_b folded in as a rank-1 (K=1) accumulation: ones.T @ hb
              nc.tensor.matmul(
                  psum_l[:], ones1[:], hb_s[:], start=False, stop=True
              )
              nc.scalar.copy(outs[:], psum_l[:])
              nc.gpsimd.dma_start(out=out[:], in_=outs[:])

    if split:
        _split_multiwaits(nc)
    return nc


_NC = None


def _get_nc():
    global _NC
    if _NC is None:
        _NC = _build_nc()
    return _NC


def make_in_maps(
    data0, data1, data2, conv_w, bn_gamma, bn_beta, bn_mean, bn_var, head_w, head_b
):
    f = np.float32
    data0 = np.ascontiguousarray(np.asarray(data0), dtype=NPBF16)
    data1 = np.ascontiguousarray(np.asarray(data1), dtype=NPBF16)
    data2 = np.ascontiguousarray(np.asarray(data2), dtype=NPBF16)
    conv_w = np.asarray(conv_w, dtype=f)
    bn_gamma = np.asarray(bn_gamma, dtype=f)
    bn_beta = np.asarray(bn_beta, dtype=f)
    bn_mean = np.asarray(bn_mean, dtype=f)
    bn_var = np.asarray(bn_var, dtype=f)
    head_w = np.asarray(head_w, dtype=f)
    head_b = np.asarray(head_b, dtype=f)

    inv = (bn_gamma / np.sqrt(bn_var + np.float32(1e-5))).astype(f)
    w_folded = (conv_w * inv[None, :]).astype(f)  # [384, 1024]
    b_folded = (bn_beta - bn_mean * inv).astype(f)  # [1024]

    # wT[p, k*1024+j] = W'[k*128+p, j]  (K-chunk-major along free dim)
    wT = np.ascontiguousarray(
        w_folded.reshape(3, 128, OUTC).transpose(1, 0, 2).reshape(128, 3 * OUTC),
        dtype=NPBF16,
    )
    # bias8[p, m] = b'[m*128+p]
    bias8 = np.ascontiguousarray(b_folded.reshape(8, 128).T)
    # hw8[p, m*40+q] = head_w[m*128+p, q] / 64   (1/64 folds the mean-pool)
    hw8 = np.ascontiguousarray(
        (head_w / np.float32(N2)).reshape(8, 128, NCLS).transpose(1, 0, 2).reshape(128, 8 * NCLS)
    )
    hb = np.ascontiguousarray(head_b.reshape(1, NCLS))

    in_maps = []
    for c in range(N_CORES):
        in_maps.append(
            {
                "d0": data0[c * D0 : (c + 1) * D0],
                "d1": data1[c * D1 : (c + 1) * D1],
                "d2": data2[c * D2 : (c + 1) * D2],
                "wT": wT,
                "bias8": bias8,
                "hw8": hw8,
                "hb": hb,
            }
        )
    return in_maps


_RUNNER = None


def _make_runner(nc):
    """Jitted SPMD executor (mirrors bass2jax.run_bass_via_pjrt but reuses
    one jit so repeated calls don't re-trace/re-compile)."""
    if True:
        import jax
        from jax.experimental.shard_map import shard_map
        from jax.sharding import Mesh, PartitionSpec

        from concourse import bass2jax, mybir as mb

        bass2jax.install_neuronx_cc_hook()
        partition_name = (
            nc.partition_id_tensor.name if nc.partition_id_tensor else None
        )
        in_names, out_names, out_avals, zero_outs = [], [], [], []
        for alloc in nc.m.functions[0].allocations:
            if not isinstance(alloc, mb.MemoryLocationSet):
                continue
            name = alloc.memorylocations[0].name
            if alloc.kind == "ExternalInput":
                if name != partition_name:
                    in_names.append(name)
            elif alloc.kind == "ExternalOutput":
                out_names.append(name)
                shape = tuple(alloc.tensor_shape)
                dtype = mb.dt.np(alloc.dtype)
                out_avals.append(jax.core.ShapedArray(shape, dtype))
                zero_outs.append(np.zeros(shape, dtype))
        n_params = len(in_names)
        all_in_names = in_names + out_names
        if partition_name is not None:
            all_in_names = all_in_names + [partition_name]

        def _body(*args):
            operands = list(args)
            if partition_name is not None:
                operands.append(bass2jax.partition_id_tensor())
            outs = bass2jax._bass_exec_p.bind(
                *operands,
                out_avals=tuple(out_avals),
                in_names=tuple(all_in_names),
                out_names=tuple(out_names),
                lowering_input_output_aliases=(),
                sim_require_finite=True,
                sim_require_nnan=True,
                nc=nc,
            )
            return tuple(outs)

        devices = jax.devices()[:N_CORES]
        mesh = Mesh(np.asarray(devices), ("core",))
        n_outs = len(out_avals)
        in_specs = (PartitionSpec("core"),) * (n_params + n_outs)
        out_specs = (PartitionSpec("core"),) * n_outs
        # No donation: the kernel writes every element of "out", so the
        # zero placeholder inputs can live on device and be reused.
        sharded = jax.jit(
            shard_map(
                _body,
                mesh=mesh,
                in_specs=in_specs,
                out_specs=out_specs,
                check_rep=False,
            ),
            keep_unused=True,
        )
        return dict(
            nc=nc,
            sharded=sharded,
            in_names=in_names,
            out_names=out_names,
            out_avals=out_avals,
            zero_outs=zero_outs,
            mesh=mesh,
        )


def _get_runner():
    global _RUNNER
    if _RUNNER is None:
        _RUNNER = _make_runner(_get_nc())
    return _RUNNER


def _concat_inputs(r, in_maps):
    return [
        np.concatenate([np.asarray(m[name]) for m in in_maps], axis=0)
        for name in r["in_names"]
    ]


def _concat_zeros(r):
    return [
        np.zeros((N_CORES * z.shape[0], *z.shape[1:]), z.dtype)
        for z in r["zero_outs"]
    ]


def _run(r, concat_in, concat_zeros=None):
    if concat_zeros is None:
        concat_zeros = _concat_zeros(r)
    out_arrs = r["sharded"](*concat_in, *concat_zeros)
    return out_arrs


def kernel(**inputs) -> np.ndarray:
    r = _get_runner()
    in_maps = make_in_maps(**inputs)
    out_arrs = _run(r, _concat_inputs(r, in_maps))
    return np.asarray(out_arrs[r["out_names"].index("out")])


def device_place_and_time(r, inputs, iters=20, batches=4):
    """Pre-place inputs on device, then time batches of back-to-back
    dispatches.  Returns (per-call seconds list, out array)."""
    import time

    import jax
    from jax.sharding import NamedSharding, PartitionSpec

    sharding = NamedSharding(r["mesh"], PartitionSpec("core"))
    concat_in = _concat_inputs(r, make_in_maps(**inputs))
    dev_in = [jax.device_put(a, sharding) for a in concat_in]
    dev_zeros = [jax.device_put(z, sharding) for z in _concat_zeros(r)]
    out_arrs = _run(r, dev_in, dev_zeros)  # warm
    out = np.asarray(out_arrs[r["out_names"].index("out")])
    times = []
    for _ in range(batches):
        t0 = time.perf_counter()
        last = None
        for _ in range(iters):
            last = _run(r, dev_in, dev_zeros)
        jax.block_until_ready(last)
        t1 = time.perf_counter()
        times.append((t1 - t0) / iters)
    return times, out
